# revision 6
# baseline (speedup 1.0000x reference)
"""Trainium2 Bass kernel for nn_CMR_59931973648949 (gnn_message_passing).

Contract: kernel(**inputs) takes FULL unsharded numpy inputs and returns the
FULL [16, 1024] output. Internally: data-parallel over batch across 8 cores
(2 samples each); the final W_out projection is model-parallel (each core owns
a 128-wide output-column slice, fed by an AllGather of the per-sample `mem`
vectors).

Math (per sample b), refactored from the reference:
  feat_vT[d,n]  = sum_v WvT[v,d] * visf[v,n] * scl[n]
                  where scl[n] = mean(norm_w) / max(||visf[:,n]||, 1e-12)
  qT[d,k]       = sum_w WnT[w,d] * node_repT[w,k]     (WnT = W_node.T/sqrt(DV))
  u0T[d,r]      = sum_w WA0[w,d] * relate_repT[w,r]   (WA0 = W_rel.T@W_e0/sqrt(DE))
  u1T[d,r]      = sum_w WA1[w,d] * relate_repT[w,r]
  logits[k,n]   = sum_d qT[d,k]*feat_vT[d,n];  find = softmax_n(mask(logits))*node_mask
  A0[r,n]       = sum_d u0T[d,r]*feat_vT[d,n];  A1T[n,r] = sum_d feat_vT[d,n]*u1T[d,r]
  ea_r[n,m]     = sigmoid(A0[r,m] + A1T[n,r]) * relation_mask[n,m]
  g_findT[n,r]  = sum_k find[k,n]*GT[k,r]   (GT folds valid*relate_mask*onehot(obj))
  h[r,m]        = sum_n g_findT[n,r]*ea_r[n,m]
  find2T[m,k']  = findT[m,k'] + sum_r h[r,m]*ST[r,k']  (ST = onehot(subj))
  fa[m] = max_k' find2T; fa /= max(max_m fa, 1);  fa = fa*bm + (1-bm)*1e-7
  mem[v] = sum_n visf[v,n]*fa[n];  out = mem @ W_out.T + b_out
"""

import numpy as np

import concourse.bass as bass
import concourse.tile as tile
from concourse import bacc, mybir
from concourse.bass_utils import run_bass_kernel_spmd

P = 128
B, K, R, N = 16, 12, 12, 64
DW, DV, DVIS, DE, DC = 512, 512, 2048, 512, 1024
NCORES = 8
S = B // NCORES  # samples per core = 2
CSLICE = DC // NCORES  # W_out output columns per core = 128

F32 = mybir.dt.float32

_cache = {}


def _pack(a):
    """[(o*128), F] row-major -> [128, o*F] partition-major."""
    o = a.shape[0] // P
    return np.ascontiguousarray(
        a.reshape(o, P, a.shape[1]).transpose(1, 0, 2).reshape(P, -1)
    )


def build_nc():
    nc = bacc.Bacc(num_devices=NCORES)

    # ---- DRAM parameters (per-core views; host packs partition-major) ----
    d_visf = nc.declare_dram_parameter("visf", [S, P, 16 * N], F32, isOutput=False)
    d_nrepT = nc.declare_dram_parameter("nrepT", [S, P, 4 * K], F32, isOutput=False)
    d_rrepT = nc.declare_dram_parameter("rrepT", [S, P, 4 * R], F32, isOutput=False)
    d_WvT = nc.declare_dram_parameter("WvT", [P, 16 * DV], F32, isOutput=False)
    d_WnT = nc.declare_dram_parameter("WnT", [P, 4 * DV], F32, isOutput=False)
    d_WA0 = nc.declare_dram_parameter("WA0", [P, 4 * DV], F32, isOutput=False)
    d_WA1 = nc.declare_dram_parameter("WA1", [P, 4 * DV], F32, isOutput=False)
    d_WoT = nc.declare_dram_parameter("WoT", [P, 16 * CSLICE], F32, isOutput=False)
    d_bout = nc.declare_dram_parameter("bout", [B, CSLICE], F32, isOutput=False)
    d_GT = nc.declare_dram_parameter("GT", [S, K, R], F32, isOutput=False)
    d_ST = nc.declare_dram_parameter("ST", [S, R, K], F32, isOutput=False)
    d_rmask = nc.declare_dram_parameter("rmask", [S, N, N], F32, isOutput=False)
    d_bmmul = nc.declare_dram_parameter("bmmul", [S, K, N], F32, isOutput=False)
    d_bmadd = nc.declare_dram_parameter("bmadd", [S, K, N], F32, isOutput=False)
    d_nmcol = nc.declare_dram_parameter("nmcol", [S, K, 1], F32, isOutput=False)
    d_famul = nc.declare_dram_parameter("famul", [S, 1, N], F32, isOutput=False)
    d_faadd = nc.declare_dram_parameter("faadd", [S, 1, N], F32, isOutput=False)
    d_I64 = nc.declare_dram_parameter("I64", [N, N], F32, isOutput=False)
    d_out = nc.declare_dram_parameter("out", [B, CSLICE], F32, isOutput=True)

    # internal DRAM for the mem AllGather (layout [vi, vo*S] per core)
    d_memloc = nc.dram_tensor("memloc", [P, 16 * S], F32)
    d_memall = nc.dram_tensor("memall", [NCORES, P, 16 * S], F32, addr_space="Shared")

    with tile.TileContext(nc) as tc:
        with (
            tc.tile_pool(name="singles", bufs=1) as singles,
            tc.tile_pool(name="ps", bufs=2) as ps,
            tc.tile_pool(name="ea", bufs=2) as eap,
            tc.tile_pool(name="psum", bufs=8, space="PSUM") as psum,
        ):
            # ---------- resident constants ----------
            WvT_sb = singles.tile([P, 16, DV], F32)
            for g in range(4):
                nc.sync.dma_start(
                    out=WvT_sb[:, 4 * g : 4 * g + 4, :],
                    in_=d_WvT[:, 4 * g * DV : 4 * (g + 1) * DV].rearrange(
                        "p (o d) -> p o d", o=4
                    ),
                )
            WnT_sb = singles.tile([P, 4, DV], F32)
            nc.sync.dma_start(
                out=WnT_sb[:], in_=d_WnT[:].rearrange("p (o d) -> p o d", o=4)
            )
            WA0_sb = singles.tile([P, 4, DV], F32)
            nc.sync.dma_start(
                out=WA0_sb[:], in_=d_WA0[:].rearrange("p (o d) -> p o d", o=4)
            )
            WA1_sb = singles.tile([P, 4, DV], F32)
            nc.sync.dma_start(
                out=WA1_sb[:], in_=d_WA1[:].rearrange("p (o d) -> p o d", o=4)
            )
            I64_sb = singles.tile([N, N], F32)
            nc.sync.dma_start(out=I64_sb[:], in_=d_I64[:])
            ones_col = singles.tile([P, 1], F32)
            nc.vector.memset(ones_col[:], 1.0)
            ones_1xN = singles.tile([1, N], F32)
            nc.vector.memset(ones_1xN[:], 1.0)
            ones_1xP = singles.tile([1, P], F32)
            nc.vector.memset(ones_1xP[:], 1.0)
            mem_sb = singles.tile([P, 16, S], F32)

            # ---------- per-sample pipeline ----------
            for s in range(S):
                visf_sb = ps.tile([P, 16, N], F32)
                nc.sync.dma_start(
                    out=visf_sb[:],
                    in_=d_visf[s].rearrange("p (o n) -> p o n", o=16),
                )
                rmask_sb = ps.tile([N, N], F32)
                nc.sync.dma_start(out=rmask_sb[:], in_=d_rmask[s])
                bmmul_sb = ps.tile([K, N], F32)
                nc.sync.dma_start(out=bmmul_sb[:], in_=d_bmmul[s])
                bmadd_sb = ps.tile([K, N], F32)
                nc.sync.dma_start(out=bmadd_sb[:], in_=d_bmadd[s])
                nmcol_sb = ps.tile([K, 1], F32)
                nc.sync.dma_start(out=nmcol_sb[:], in_=d_nmcol[s])
                famul_sb = ps.tile([1, N], F32)
                nc.sync.dma_start(out=famul_sb[:], in_=d_famul[s])
                faadd_sb = ps.tile([1, N], F32)
                nc.sync.dma_start(out=faadd_sb[:], in_=d_faadd[s])
                GT_sb = ps.tile([K, R], F32)
                nc.sync.dma_start(out=GT_sb[:], in_=d_GT[s])
                ST_sb = ps.tile([R, K], F32)
                nc.sync.dma_start(out=ST_sb[:], in_=d_ST[s])
                nrepT_sb = ps.tile([P, 4, K], F32)
                nc.sync.dma_start(
                    out=nrepT_sb[:], in_=d_nrepT[s].rearrange("p (o k) -> p o k", o=4)
                )
                rrepT_sb = ps.tile([P, 4, R], F32)
                nc.sync.dma_start(
                    out=rrepT_sb[:], in_=d_rrepT[s].rearrange("p (o k) -> p o k", o=4)
                )

                # -- column norms of visf: scl[n] = 1/max(||visf[:,n]||, 1e-12)
                sq_sb = ps.tile([P, 16, N], F32)
                nc.scalar.square(out=sq_sb[:], in_=visf_sb[:])
                presum = ps.tile([P, N], F32)
                nc.vector.tensor_reduce(
                    out=presum[:],
                    in_=sq_sb[:].rearrange("p o n -> p n o"),
                    axis=mybir.AxisListType.X,
                    op=mybir.AluOpType.add,
                )
                sqn_ps = psum.tile([N, 1], F32, tag="ps")
                nc.tensor.matmul(
                    out=sqn_ps[:], lhsT=presum[:], rhs=ones_col[:], start=True, stop=True
                )
                scl = ps.tile([N, 1], F32)
                nc.scalar.sqrt(out=scl[:], in_=sqn_ps[:])
                nc.vector.tensor_scalar_max(out=scl[:], in0=scl[:], scalar1=1e-12)
                nc.vector.reciprocal(out=scl[:], in_=scl[:])

                # -- feat_v [n, d] then transpose to feat_vT [d, n]
                featv_ps = psum.tile([N, DV], F32, tag="ps")
                for c in range(16):
                    nc.tensor.matmul(
                        out=featv_ps[:],
                        lhsT=visf_sb[:, c, :],
                        rhs=WvT_sb[:, c, :],
                        start=(c == 0),
                        stop=(c == 15),
                    )
                featv_sb = ps.tile([N, DV], F32)
                nc.vector.tensor_scalar_mul(
                    out=featv_sb[:], in0=featv_ps[:], scalar1=scl[:]
                )
                ftT_ps = psum.tile([P, 4, N], F32, tag="ps")
                for c in range(4):
                    nc.tensor.matmul(
                        out=ftT_ps[:, c, :],
                        lhsT=featv_sb[:, P * c : P * (c + 1)],
                        rhs=I64_sb[:],
                        start=(c == 0),
                        stop=(c == 3),
                    )
                ftT_sb = ps.tile([P, 4, N], F32)
                nc.vector.tensor_copy(out=ftT_sb[:], in_=ftT_ps[:])

                # -- qT/u0T/u1T [d, 12] via 16 small matmuls each
                def lin_T(w_sb, x_sb, ncols, name):
                    out_ps = psum.tile([P, 4, ncols], F32, tag="ps", name=name + "_ps")
                    for dc in range(4):
                        for wc in range(4):
                            nc.tensor.matmul(
                                out=out_ps[:, dc, :],
                                lhsT=w_sb[:, wc, P * dc : P * (dc + 1)],
                                rhs=x_sb[:, wc, :],
                                start=(dc == 0 and wc == 0),
                                stop=(dc == 3 and wc == 3),
                            )
                    out_sb = ps.tile([P, 4, ncols], F32, name=name)
                    nc.scalar.copy(out=out_sb[:], in_=out_ps[:])
                    return out_sb

                qT_sb = lin_T(WnT_sb, nrepT_sb, K, "qT")
                u0T_sb = lin_T(WA0_sb, rrepT_sb, R, "u0T")
                u1T_sb = lin_T(WA1_sb, rrepT_sb, R, "u1T")

                # -- node attention: logits -> masked softmax -> find [k, n]
                logits_ps = psum.tile([K, N], F32, tag="ps")
                for c in range(4):
                    nc.tensor.matmul(
                        out=logits_ps[:],
                        lhsT=qT_sb[:, c, :],
                        rhs=ftT_sb[:, c, :],
                        start=(c == 0),
                        stop=(c == 3),
                    )
                lg_sb = ps.tile([K, N], F32)
                nc.vector.tensor_tensor(
                    out=lg_sb[:], in0=logits_ps[:], in1=bmmul_sb[:],
                    op=mybir.AluOpType.mult,
                )
                nc.vector.tensor_tensor(
                    out=lg_sb[:], in0=lg_sb[:], in1=bmadd_sb[:],
                    op=mybir.AluOpType.add,
                )
                nmx = ps.tile([K, 1], F32)
                nc.vector.tensor_reduce(
                    out=nmx[:], in_=lg_sb[:], axis=mybir.AxisListType.X,
                    op=mybir.AluOpType.max, negate=True,
                )
                e_sb = ps.tile([K, N], F32)
                ssum = ps.tile([K, 1], F32)
                nc.scalar.activation(
                    out=e_sb[:], in_=lg_sb[:],
                    func=mybir.ActivationFunctionType.Exp,
                    bias=nmx[:], scale=1.0, accum_out=ssum[:],
                )
                rs = ps.tile([K, 1], F32)
                nc.vector.reciprocal(out=rs[:], in_=ssum[:])
                nc.vector.tensor_tensor(
                    out=rs[:], in0=rs[:], in1=nmcol_sb[:], op=mybir.AluOpType.mult
                )
                find_sb = ps.tile([K, N], F32)
                nc.vector.tensor_scalar_mul(out=find_sb[:], in0=e_sb[:], scalar1=rs[:])

                # -- g_findT [n, r]; open find2T accumulation with findT
                gfT_ps = psum.tile([N, R], F32, tag="ps")
                nc.tensor.matmul(
                    out=gfT_ps[:], lhsT=find_sb[:], rhs=GT_sb[:], start=True, stop=True
                )
                gfT_sb = ps.tile([N, R], F32)
                nc.scalar.copy(out=gfT_sb[:], in_=gfT_ps[:])
                f2T_ps = psum.tile([N, K], F32, tag="ps")
                nc.tensor.matmul(
                    out=f2T_ps[:], lhsT=find_sb[:], rhs=I64_sb[:K, :K],
                    start=True, stop=False,
                )

                # -- A0 [r, n] and A1T [n, r]
                A0_ps = psum.tile([R, N], F32, tag="ps")
                for c in range(4):
                    nc.tensor.matmul(
                        out=A0_ps[:], lhsT=u0T_sb[:, c, :], rhs=ftT_sb[:, c, :],
                        start=(c == 0), stop=(c == 3),
                    )
                A0_sb = ps.tile([R, N], F32)
                nc.scalar.copy(out=A0_sb[:], in_=A0_ps[:])
                A1T_ps = psum.tile([N, R], F32, tag="ps")
                for c in range(4):
                    nc.tensor.matmul(
                        out=A1T_ps[:], lhsT=ftT_sb[:, c, :], rhs=u1T_sb[:, c, :],
                        start=(c == 0), stop=(c == 3),
                    )
                A1T_sb = ps.tile([N, R], F32)
                nc.scalar.copy(out=A1T_sb[:], in_=A1T_ps[:])

                # -- edge attention + h, per relation r
                ea_all = ps.tile([N, R, N], F32)
                hT_ps = psum.tile([N, R], F32, tag="ps")
                for r in range(R):
                    B_ps = psum.tile([N, N], F32, tag="ps", name="B_ps")
                    # B_ps[i, j] = A0[r, j]: lhsT column r of I, broadcast over i
                    nc.tensor.matmul(
                        out=B_ps[:],
                        lhsT=I64_sb[:K, r : r + 1].to_broadcast([K, N]),
                        rhs=A0_sb[:],
                        start=True, stop=True,
                    )
                    nc.scalar.activation(
                        out=ea_all[:, r, :], in_=B_ps[:],
                        func=mybir.ActivationFunctionType.Sigmoid,
                        bias=A1T_sb[:, r : r + 1], scale=1.0,
                    )
                nc.vector.tensor_tensor(
                    out=ea_all[:],
                    in0=ea_all[:],
                    in1=rmask_sb[:, None, :].to_broadcast([N, R, N]),
                    op=mybir.AluOpType.mult,
                )
                for r in range(R):
                    nc.tensor.matmul(
                        out=hT_ps[:, r : r + 1],
                        lhsT=ea_all[:, r, :],
                        rhs=gfT_sb[:, r : r + 1],
                        start=(r == 0),
                        stop=(r == R - 1),
                    )
                hT_sb = ps.tile([N, R], F32)
                nc.scalar.copy(out=hT_sb[:], in_=hT_ps[:])
                h_ps = psum.tile([R, N], F32, tag="ps")
                nc.tensor.matmul(
                    out=h_ps[:], lhsT=hT_sb[:], rhs=I64_sb[:], start=True, stop=True
                )
                h_sb = ps.tile([R, N], F32)
                nc.scalar.copy(out=h_sb[:], in_=h_ps[:])

                # -- find2T = findT + h.T @ ST ; final attention
                nc.tensor.matmul(
                    out=f2T_ps[:], lhsT=h_sb[:], rhs=ST_sb[:], start=False, stop=True
                )
                fa_sb = ps.tile([N, 1], F32)
                nc.vector.tensor_reduce(
                    out=fa_sb[:], in_=f2T_ps[:], axis=mybir.AxisListType.X,
                    op=mybir.AluOpType.max,
                )
                faT_ps = psum.tile([1, N], F32, tag="ps")
                nc.tensor.matmul(
                    out=faT_ps[:], lhsT=fa_sb[:], rhs=I64_sb[:], start=True, stop=True
                )
                nr = ps.tile([1, 1], F32)
                nc.vector.tensor_reduce(
                    out=nr[:], in_=faT_ps[:], axis=mybir.AxisListType.X,
                    op=mybir.AluOpType.max,
                )
                nc.vector.tensor_scalar_max(out=nr[:], in0=nr[:], scalar1=1.0)
                nc.vector.reciprocal(out=nr[:], in_=nr[:])
                faT_sb = ps.tile([1, N], F32)
                nc.vector.tensor_scalar_mul(out=faT_sb[:], in0=faT_ps[:], scalar1=nr[:])
                nc.vector.tensor_tensor(
                    out=faT_sb[:], in0=faT_sb[:], in1=famul_sb[:],
                    op=mybir.AluOpType.mult,
                )
                nc.vector.tensor_tensor(
                    out=faT_sb[:], in0=faT_sb[:], in1=faadd_sb[:],
                    op=mybir.AluOpType.add,
                )

                # -- mem[v] = sum_n visf[v, n] * fa[n]
                fabc_ps = psum.tile([P, N], F32, tag="ps")
                nc.tensor.matmul(
                    out=fabc_ps[:], lhsT=ones_1xP[:], rhs=faT_sb[:],
                    start=True, stop=True,
                )
                wtmp = ps.tile([P, 16, N], F32, tag="sq_sb")
                nc.vector.tensor_tensor(
                    out=wtmp[:],
                    in0=visf_sb[:],
                    in1=fabc_ps[:, None, :].to_broadcast([P, 16, N]),
                    op=mybir.AluOpType.mult,
                )
                nc.vector.tensor_reduce(
                    out=mem_sb[:, :, s], in_=wtmp[:], axis=mybir.AxisListType.X,
                    op=mybir.AluOpType.add,
                )

            # ---------- AllGather mem and W_out slice matmul ----------
            WoT_sb = singles.tile([P, 16, CSLICE], F32)
            for g in range(4):
                nc.sync.dma_start(
                    out=WoT_sb[:, 4 * g : 4 * g + 4, :],
                    in_=d_WoT[:, 4 * g * CSLICE : 4 * (g + 1) * CSLICE].rearrange(
                        "p (o d) -> p o d", o=4
                    ),
                )
            bout_sb = singles.tile([B, CSLICE], F32)
            nc.sync.dma_start(out=bout_sb[:], in_=d_bout[:])

            nc.sync.dma_start(out=d_memloc[:], in_=mem_sb[:].rearrange("p o s -> p (o s)"))
            nc.gpsimd.collective_compute(
                "AllGather",
                mybir.AluOpType.bypass,
                replica_groups=[list(range(NCORES))],
                ins=[d_memloc[:]],
                outs=[d_memall[:]],
            )
            memall_sb = singles.tile([P, 16, B], F32)
            nc.sync.dma_start(
                out=memall_sb[:].rearrange("p o (c s) -> p o c s", c=NCORES),
                in_=d_memall[:].rearrange("c p (o s) -> p o c s", o=16),
            )
            out_ps = psum.tile([B, CSLICE], F32, tag="ps")
            for c in range(16):
                nc.tensor.matmul(
                    out=out_ps[:],
                    lhsT=memall_sb[:, c, :],
                    rhs=WoT_sb[:, c, :],
                    start=(c == 0),
                    stop=(c == 15),
                )
            out_sb = singles.tile([B, CSLICE], F32)
            nc.vector.tensor_tensor(
                out=out_sb[:], in0=out_ps[:], in1=bout_sb[:], op=mybir.AluOpType.add
            )
            nc.sync.dma_start(out=d_out[:], in_=out_sb[:])

    nc.finalize()
    return nc


def _host_prep(inputs):
    node_rep = np.asarray(inputs["node_rep"], np.float32)
    relate_rep = np.asarray(inputs["relate_rep"], np.float32)
    relate_os = np.asarray(inputs["relate_os"])
    relate_mask = np.asarray(inputs["relate_mask"], np.float32)
    vision_feat = np.asarray(inputs["vision_feat"], np.float32)
    relation_mask = np.asarray(inputs["relation_mask"], np.float32)
    box_mask = np.asarray(inputs["box_mask"], np.float32)
    node_mask = np.asarray(inputs["node_mask"], np.float32)
    norm_w = np.asarray(inputs["norm_w"], np.float32)
    W_v = np.asarray(inputs["W_v"], np.float32)
    W_e = np.asarray(inputs["W_e"], np.float32)
    W_node = np.asarray(inputs["W_node"], np.float32)
    W_rel = np.asarray(inputs["W_rel"], np.float32)
    W_out = np.asarray(inputs["W_out"], np.float32)
    b_out = np.asarray(inputs["b_out"], np.float32)

    s_mean = np.float32(np.mean(norm_w))
    WvT = (W_v.T * s_mean).astype(np.float32)  # [DVIS, DV]
    WnT = (W_node.T / np.float32(np.sqrt(DV))).astype(np.float32)  # [DW, DV]
    WA0 = (W_rel.T @ W_e[:, :DV] / np.float32(np.sqrt(DE))).astype(np.float32)
    WA1 = (W_rel.T @ W_e[:, DV:] / np.float32(np.sqrt(DE))).astype(np.float32)
    WoT = np.ascontiguousarray(W_out.T)  # [DVIS, DC]

    subj = relate_os[..., 1].astype(np.int64)  # [B, R]
    obj = relate_os[..., 0].astype(np.int64)
    valid = (subj != -1).astype(np.float32)
    obj_c = np.clip(obj, 0, K - 1)
    subj_c = np.clip(subj, 0, K - 1)
    G = np.zeros((B, R, K), np.float32)  # gather one-hot * valid * relate_mask
    STm = np.zeros((B, R, K), np.float32)  # scatter one-hot
    bi = np.arange(B)[:, None]
    ri = np.arange(R)[None, :]
    G[bi, ri, obj_c] = valid * relate_mask
    STm[bi, ri, subj_c] = 1.0

    bmmul = (box_mask > 0).astype(np.float32)  # [B, N]
    bmadd = (bmmul - 1.0) * np.float32(1e9)
    famul = box_mask
    faadd = (1.0 - box_mask) * np.float32(1e-7)

    WvT_p = _pack(WvT)
    WnT_p = _pack(WnT)
    WA0_p = _pack(WA0)
    WA1_p = _pack(WA1)

    in_maps = []
    for c in range(NCORES):
        bs = slice(S * c, S * (c + 1))
        cs = slice(CSLICE * c, CSLICE * (c + 1))
        m = {
            "visf": np.stack([_pack(vision_feat[b]) for b in range(S * c, S * c + S)]),
            "nrepT": np.stack(
                [_pack(np.ascontiguousarray(node_rep[b].T)) for b in range(S * c, S * c + S)]
            ),
            "rrepT": np.stack(
                [_pack(np.ascontiguousarray(relate_rep[b].T)) for b in range(S * c, S * c + S)]
            ),
            "WvT": WvT_p,
            "WnT": WnT_p,
            "WA0": WA0_p,
            "WA1": WA1_p,
            "WoT": _pack(np.ascontiguousarray(WoT[:, cs])),
            "bout": np.ascontiguousarray(
                np.broadcast_to(b_out[cs][None, :], (B, CSLICE))
            ).astype(np.float32),
            "GT": np.ascontiguousarray(G[bs].transpose(0, 2, 1)),
            "ST": np.ascontiguousarray(STm[bs]),
            "rmask": np.ascontiguousarray(relation_mask[bs]),
            "bmmul": np.ascontiguousarray(
                np.broadcast_to(bmmul[bs, None, :], (S, K, N))
            ),
            "bmadd": np.ascontiguousarray(
                np.broadcast_to(bmadd[bs, None, :], (S, K, N))
            ),
            "nmcol": np.ascontiguousarray(node_mask[bs][:, :, None]),
            "famul": np.ascontiguousarray(famul[bs][:, None, :]),
            "faadd": np.ascontiguousarray(faadd[bs][:, None, :]),
            "I64": np.eye(N, dtype=np.float32),
        }
        in_maps.append(m)
    return in_maps


def kernel(**inputs) -> np.ndarray:
    if "nc" not in _cache:
        _cache["nc"] = build_nc()
    nc = _cache["nc"]
    in_maps = _host_prep(inputs)
    res = run_bass_kernel_spmd(nc, in_maps, core_ids=list(range(NCORES)))
    outs = [res.results[c]["out"] for c in range(NCORES)]
    return np.concatenate(outs, axis=1).astype(np.float32)
